# revision 17
# baseline (speedup 1.0000x reference)
"""GQA attention (B=4, S=1024, H=4096, 32 q heads / 8 kv heads, rotary) on 8 trn2 cores.

Sharding: DP4 x TP2. Core c = 2*b + j handles batch b with kv-head half j:
  - column-parallel wq/wk/wv (16 q heads / 4 kv heads per core)
  - row-parallel wo -> partial outputs, host sums core pairs.

v4: all-bf16 dataflow, fully SBUF-resident (no DRAM spill round trips),
with the ACT-bound attention interleaved under the PE-bound projections:

  prologue: project k (4 blocks), v (4, PE-transposed to [t,d]),
    q head-group 0. Rope pair-mix = partition-half swap via a
    half-swapped-identity PE matmul, deferred one block.
  ready-queue interleave: after each q projection block, up to two
    attention chunks (head, s-half) whose q block's rope flushed two
    blocks earlier are emitted, so exp/DVE softmax work hides under
    projection matmuls from the earliest possible block; only the last
    4 chunks drain after projections end. Attention chunk: scoresT =
    kT.T @ qT into 1-bank [128,512] PSUM, exp on ACT (1/sqrt(D) folded
    into the activation scale), DVE pairwise tree over tb + ones(128x128)
    matmul = partition-reduce-and-broadcast, wide reciprocal,
    oT = (v.T @ expT) * invb; denb/recip/norm deferred one chunk.
  phase 3: outT[h,s] = wo.T @ oT (stationary wo tile streams all 1024
    s-cols, 16-step co accumulation); host transposes outT.
"""

import numpy as np

B = 4
S = 1024
H = 4096
D = 128
HQ = 32
HKV = 8
G = 4
NCORES = 8
QC = 2048  # q cols per core
KC = 512  # k cols per core
VC = 512  # v cols per core
COH = 2048  # wo rows per core
ROPE_BASE = 10000.0

_CACHE = {}


def _build(reps=1):
    import concourse.tile as tile
    from concourse import bacc, mybir
    from concourse.masks import make_identity

    fp32 = mybir.dt.float32
    bf16 = mybir.dt.bfloat16

    nc = bacc.Bacc(None, target_bir_lowering=False)

    xT_d = nc.dram_tensor("xT", [H, S], bf16, kind="ExternalInput")
    wq_d = nc.dram_tensor("wq", [H, QC], bf16, kind="ExternalInput")
    wk_d = nc.dram_tensor("wk", [H, KC], bf16, kind="ExternalInput")
    wv_d = nc.dram_tensor("wv", [H, VC], bf16, kind="ExternalInput")
    wo_d = nc.dram_tensor("wo", [COH, H], bf16, kind="ExternalInput")
    am_d = nc.dram_tensor("ropeA", [D, 2, 512], bf16, kind="ExternalInput")
    bm_d = nc.dram_tensor("ropeB", [D, 2, 512], bf16, kind="ExternalInput")
    outT_d = nc.dram_tensor("outT", [32, D, S], bf16, kind="ExternalOutput")

    xT_r = xT_d.rearrange("(ko p) t -> p ko t", p=128)  # [128, 32, 1024]
    wq_r = wq_d.rearrange("(ko p) c -> p ko c", p=128)
    wk_r = wk_d.rearrange("(ko p) c -> p ko c", p=128)
    wv_r = wv_d.rearrange("(ko p) c -> p ko c", p=128)
    wo_r = wo_d.rearrange("(co p) h -> p co h", p=128)  # [128, 16, 4096]

    NKO = H // 128  # 32 contraction tiles
    SC_EXP = float(1.0 / np.sqrt(128.0))

    with tile.TileContext(nc) as tc, nc.allow_low_precision(
        reason="bf16 matmul pipeline, tolerance 2e-2"
    ):
      for _rep in range(reps):
        with (
            tc.tile_pool(name="persist", bufs=1) as persist,
            tc.tile_pool(name="konst", bufs=1) as konst,
            tc.tile_pool(name="qt", bufs=2) as qpool,
        ):
            kT = persist.tile([128, HKV // 2, 2, 512], bf16)  # [d, kvh, sh, si]
            v = persist.tile([128, S // 128, VC], bf16)  # [t, tb, vcols]
            oT = persist.tile([128, 16, 2, 512], bf16)  # [d, qh, sh, si]
            ones128 = konst.tile([128, 128], bf16)
            nc.vector.memset(ones128[:], 1.0)
            pswap = konst.tile([128, 128], bf16)

            # ---------------- attention machinery ----------------
            with (
                tc.tile_pool(name="ex", bufs=2) as expool,
                tc.tile_pool(name="sm1", bufs=1) as smpool1,
                tc.tile_pool(name="sm2", bufs=2) as smpool2,
                tc.tile_pool(name="pssc", bufs=2, space="PSUM") as pssc,
                tc.tile_pool(name="pso", bufs=2, space="PSUM") as pso,
            ):
                pending2 = []

                def flush_norm():
                    if not pending2:
                        return
                    dpart, po, cb0, sh0 = pending2.pop()
                    denb = pssc.tile([128, 512], fp32, tag="psc", name="denb")
                    nc.tensor.matmul(
                        denb[:], ones128[:], dpart[:], start=True, stop=True
                    )
                    invb = smpool2.tile([128, 512], fp32, tag="invb", name="invb")
                    nc.vector.reciprocal(invb[:], denb[:])
                    nc.vector.tensor_mul(oT[:, cb0, sh0, :], po[:], invb[:])

                def attn_chunk(qtile, h, g, sh):
                    """scores+exp+av+tree for one (head, s-half); denb/norm
                    deferred one chunk."""
                    cb = h * 4 + g
                    expT = expool.tile([128, 8, 512], bf16, tag="expT", name="expT")
                    for tb in range(8):
                        psc = pssc.tile([128, 512], fp32, tag="psc", name="psc")
                        kstat = kT[:, h, tb // 4,
                                   (tb % 4) * 128 : (tb % 4) * 128 + 128]
                        nc.tensor.matmul(
                            psc[:], kstat, qtile[:, g, sh, :],
                            start=True, stop=True,
                        )
                        nc.scalar.activation(
                            expT[:, tb], psc[:],
                            mybir.ActivationFunctionType.Exp, scale=SC_EXP,
                        )
                    po = pso.tile([128, 512], fp32, tag="po", name="po")
                    for tb in range(8):
                        vstat = v[:, tb, h * 128 : (h + 1) * 128]
                        nc.tensor.matmul(
                            po[:], vstat, expT[:, tb],
                            start=(tb == 0), stop=(tb == 7),
                        )
                    s4 = []
                    for j in range(4):
                        t = smpool1.tile([128, 512], bf16, tag=f"s4_{j}", name="s4")
                        nc.vector.tensor_add(
                            t[:], expT[:, 2 * j], expT[:, 2 * j + 1]
                        )
                        s4.append(t)
                    s2 = []
                    for j in range(2):
                        t = smpool1.tile([128, 512], bf16, tag=f"s2_{j}", name="s2")
                        nc.vector.tensor_add(t[:], s4[2 * j][:], s4[2 * j + 1][:])
                        s2.append(t)
                    dpart = smpool2.tile([128, 512], bf16, tag="dpart", name="dpart")
                    nc.vector.tensor_add(dpart[:], s2[0][:], s2[1][:])
                    flush_norm()
                    pending2.append((dpart, po, cb, sh))

                # ---------------- projections (+ rope) ----------------
                with (
                    tc.tile_pool(name="maps", bufs=1) as mpool,
                    tc.tile_pool(name="xt", bufs=8) as xpool,
                    tc.tile_pool(name="wt", bufs=2) as wpool,
                    tc.tile_pool(name="ev", bufs=2) as epool,
                    tc.tile_pool(name="kon2", bufs=1) as kon2,
                    tc.tile_pool(name="ps1", bufs=1, space="PSUM") as ps1,
                    tc.tile_pool(name="pst", bufs=1, space="PSUM") as pst,
                ):
                    ident = kon2.tile([128, 128], bf16)
                    make_identity(nc, ident[:])
                    nc.sync.dma_start(pswap[0:64, :], ident[64:128, :])
                    nc.sync.dma_start(pswap[64:128, :], ident[0:64, :])

                    def load_wt(w_r, cb, pieces=1):
                        wt = wpool.tile([128, NKO, 128], bf16, tag="wt", name="wt")
                        np_ = NKO // pieces
                        for p in range(pieces):
                            nc.sync.dma_start(
                                wt[:, p * np_ : (p + 1) * np_, :],
                                w_r[:, p * np_ : (p + 1) * np_,
                                    cb * 128 : (cb + 1) * 128],
                            )
                        return wt

                    def load_xt(ch, pieces=1):
                        xt = xpool.tile([128, 4, S], bf16, tag="xt", name=f"x{ch}")
                        np_ = 4 // pieces
                        for p in range(pieces):
                            nc.sync.dma_start(
                                xt[:, p * np_ : (p + 1) * np_, :],
                                xT_r[:, ch * 4 + p * np_ :
                                     ch * 4 + (p + 1) * np_, :],
                            )
                        return xt

                    # schedule: k(4), v(4), then q by head group
                    sched = (
                        [(wk_r, cb, "k", None) for cb in range(4)]
                        + [(wv_r, cb, "v", None) for cb in range(4)]
                        + [(wq_r, 4 * hh + j, "q", (hh, j))
                           for hh in range(4) for j in range(4)]
                    )

                    xts = [load_xt(0, pieces=4)]
                    wt_next = [load_wt(*sched[0][:2], pieces=4)]
                    maps = {}
                    for nm, dram in (("A", am_d), ("B", bm_d)):
                        mt = mpool.tile([128, 2, 512], bf16, name=nm)
                        nc.sync.dma_start(mt[:], dram[:])
                        maps[nm] = mt
                    for ch in range(1, 8):
                        xts.append(load_xt(ch))
                    Am, Bm = maps["A"], maps["B"]

                    pending = []
                    cursor = [0]  # next attention chunk (h*8 + g*2 + sh)
                    qtiles = {}  # live q tiles by head group

                    def flush_evict():
                        if not pending:
                            return
                        raw, t1, dst = pending.pop()
                        for sh in range(2):
                            psw = ps1.tile([128, 512], fp32, tag="psw", name="psw")
                            nc.tensor.matmul(
                                psw[:], pswap[:], raw[:, sh, :],
                                start=True, stop=True,
                            )
                            sw = epool.tile([128, 512], bf16, tag="sw", name="sw")
                            nc.scalar.copy(sw[:], psw[:])
                            t2 = epool.tile([128, 512], bf16, tag="t2", name="t2")
                            nc.vector.tensor_mul(t2[:], sw[:], Bm[:, sh, :])
                            nc.vector.tensor_add(dst[sh], t1[:, sh, :], t2[:])

                    for i, (w_r, cb, kind, hq) in enumerate(sched):
                        wt = wt_next[0]
                        ps = ps1.tile([128, 2, 512], fp32, tag="ps1", name="ps")
                        for ko in range(NKO):
                            xt = xts[ko // 4]
                            for sh in range(2):
                                nc.tensor.matmul(
                                    ps[:, sh, :], wt[:, ko, :],
                                    xt[:, ko % 4, sh * 512 : sh * 512 + 512],
                                    start=(ko == 0), stop=(ko == NKO - 1),
                                )
                        if i + 1 < len(sched):
                            wt_next[0] = load_wt(*sched[i + 1][:2])

                        if kind == "v":
                            raw = epool.tile(
                                [128, 1024], bf16, tag="raw", name="vraw"
                            )
                            nc.scalar.copy(raw[:], ps[:, :, :])
                            flush_evict()
                            for tb in range(8):
                                pt = pst.tile([128, 128], bf16, tag="pst", name="pt")
                                nc.tensor.transpose(
                                    pt[:], raw[:, tb * 128 : (tb + 1) * 128],
                                    ident[:],
                                )
                                nc.vector.tensor_copy(
                                    v[:, tb, cb * 128 : (cb + 1) * 128], pt[:]
                                )
                            continue

                        # q/k rope: immediate part; swap part deferred
                        raw = epool.tile([128, 2, 512], bf16, tag="raw", name="raw")
                        nc.scalar.copy(raw[:], ps[:, :, :])
                        t1 = epool.tile([128, 2, 512], bf16, tag="t1", name="t1")
                        nc.vector.tensor_mul(t1[:], raw[:], Am[:])
                        if kind == "k":
                            dst = [kT[:, cb, sh, :] for sh in range(2)]
                        else:
                            hh, j = hq
                            if j == 0:
                                qtiles[hh] = qpool.tile(
                                    [128, 4, 2, 512], bf16, tag="qt", name="qt"
                                )
                            dst = [qtiles[hh][:, j, sh, :] for sh in range(2)]
                        flush_evict()
                        pending.append((raw, t1, dst))

                        # interleave: emit up to 2 attention chunks whose
                        # q block's rope flush has completed (2 blocks ago)
                        if kind == "q":
                            blk = 8 + 4 * hq[0] + hq[1]
                            n_emit = 0
                            while cursor[0] < 32 and n_emit < 2:
                                c = cursor[0]
                                h2, r = divmod(c, 8)
                                g2, sh2 = divmod(r, 2)
                                if 8 + 4 * h2 + g2 + 2 > blk:
                                    break
                                attn_chunk(qtiles[h2], h2, g2, sh2)
                                cursor[0] += 1
                                n_emit += 1
                    flush_evict()

                # ---------- Phase 3 inside attn pools: overlap the 4-chunk
                # drain with the first two output blocks' early accumulation
                with (
                    tc.tile_pool(name="wot", bufs=2) as wopool,
                    tc.tile_pool(name="outp", bufs=2) as outpool,
                    tc.tile_pool(name="psout", bufs=2, space="PSUM") as psout,
                ):
                    def wo_dma(hb):
                        wot = wopool.tile(
                            [128, 16, 128], bf16, tag="wot", name="wot"
                        )
                        nc.sync.dma_start(
                            wot[:], wo_r[:, :, hb * 128 : (hb + 1) * 128]
                        )
                        return wot

                    def drain_one():
                        c = cursor[0]
                        h2, r = divmod(c, 8)
                        g2, sh2 = divmod(r, 2)
                        attn_chunk(qtiles[h2], h2, g2, sh2)
                        cursor[0] += 1

                    def hb_early(wot):
                        """co 0..11 (head groups 0-2, already normalized)."""
                        psO = psout.tile(
                            [128, 2, 512], fp32, tag="psO", name="psO"
                        )
                        for co in range(12):
                            for sh in range(2):
                                nc.tensor.matmul(
                                    psO[:, sh, :], wot[:, co, :],
                                    oT[:, co, sh, :],
                                    start=(co == 0), stop=False,
                                )
                        return psO

                    def hb_finish(hb, psO, wot, first_co):
                        for co in range(first_co, 16):
                            for sh in range(2):
                                nc.tensor.matmul(
                                    psO[:, sh, :], wot[:, co, :],
                                    oT[:, co, sh, :],
                                    start=False, stop=(co == 15),
                                )
                        ot = outpool.tile([128, 1024], bf16, tag="ot", name="ot")
                        nc.scalar.copy(ot[:], psO[:, :, :])
                        nc.sync.dma_start(outT_d[hb], ot[:])

                    wo_next = [wo_dma(0), wo_dma(1)]
                    drain_one(); drain_one()
                    psO0 = hb_early(wo_next[0])
                    drain_one(); drain_one()
                    psO1 = hb_early(wo_next[1])
                    flush_norm()
                    hb_finish(0, psO0, wo_next[0], 12)
                    wo_next[0] = wo_dma(2)
                    hb_finish(1, psO1, wo_next[1], 12)
                    wo_next[1] = wo_dma(3)

                    for hb in range(2, 32):
                        wot = wo_next[hb % 2]
                        psO = psout.tile(
                            [128, 2, 512], fp32, tag="psO", name="psO"
                        )
                        for co in range(16):
                            for sh in range(2):
                                nc.tensor.matmul(
                                    psO[:, sh, :], wot[:, co, :],
                                    oT[:, co, sh, :],
                                    start=(co == 0), stop=(co == 15),
                                )
                            if co == 0 and hb + 2 < 32:
                                wo_next[hb % 2] = wo_dma(hb + 2)
                        ot = outpool.tile([128, 1024], bf16, tag="ot", name="ot")
                        nc.scalar.copy(ot[:], psO[:, :, :])
                        nc.sync.dma_start(outT_d[hb], ot[:])

    nc.compile()
    return nc


def _host_prep(x, wq, wk, wv, wo, start_pos):
    import ml_dtypes

    bf16 = ml_dtypes.bfloat16
    x = np.asarray(x, dtype=np.float32)
    wq = np.asarray(wq, dtype=np.float32)
    wk = np.asarray(wk, dtype=np.float32)
    wv = np.asarray(wv, dtype=np.float32)
    wo = np.asarray(wo, dtype=np.float32)
    sp = int(np.asarray(start_pos))

    perm = np.concatenate([np.arange(0, 128, 2), np.arange(1, 128, 2)])

    def permute_cols(w):
        n = w.shape[1]
        return np.ascontiguousarray(
            w.reshape(H, n // 128, 128)[:, :, perm].reshape(H, n)
        )

    inv_freq = 1.0 / (ROPE_BASE ** (np.arange(0, D, 2, dtype=np.float32) / D))
    t = np.arange(sp, sp + S, dtype=np.float32)
    freqs = t[None, :] * inv_freq[:, None]  # [64, S]
    sin, cos = np.sin(freqs), np.cos(freqs)
    A = np.concatenate([sin, sin], axis=0).astype(np.float32)  # [128, S]
    Bm = np.concatenate([-cos, cos], axis=0).astype(np.float32)
    maps = {
        "ropeA": np.ascontiguousarray(A.reshape(D, 2, 512)).astype(bf16),
        "ropeB": np.ascontiguousarray(Bm.reshape(D, 2, 512)).astype(bf16),
    }

    in_maps = []
    for c in range(NCORES):
        b, j = divmod(c, 2)
        im = {
            "xT": np.ascontiguousarray(x[b].T).astype(bf16),
            "wq": permute_cols(wq[:, j * QC : (j + 1) * QC]).astype(bf16),
            "wk": permute_cols(wk[:, j * KC : (j + 1) * KC]).astype(bf16),
            "wv": np.ascontiguousarray(wv[:, j * VC : (j + 1) * VC]).astype(bf16),
            "wo": np.ascontiguousarray(wo[j * COH : (j + 1) * COH, :]).astype(bf16),
        }
        im.update(maps)
        in_maps.append(im)
    return in_maps


def kernel(x, wq, wk, wv, wo, start_pos=0, _trace=False):
    from concourse.bass_utils import run_bass_kernel_spmd

    if "nc" not in _CACHE:
        _CACHE["nc"] = _build()
    nc = _CACHE["nc"]

    in_maps = _host_prep(x, wq, wk, wv, wo, start_pos)
    res = run_bass_kernel_spmd(nc, in_maps, core_ids=list(range(NCORES)), trace=_trace)
    _CACHE["last_result"] = res

    out = np.empty((B, S, H), dtype=np.float32)
    for b in range(B):
        o0 = res.results[2 * b]["outT"].astype(np.float32).reshape(H, S)
        o1 = res.results[2 * b + 1]["outT"].astype(np.float32).reshape(H, S)
        out[b] = (o0 + o1).T
    return out
